# revision 8
# baseline (speedup 1.0000x reference)
"""Trainium2 Bass kernel for nn_C_Cross_Attention3D (B=16, C=768, H=W=64, HEADS=12).

Math (per batch b):
  q   = l2norm_per_head(Wq @ y_b + bq)                      # [12, 64]
  k   = Wk @ x_b + bk                                       # [768, N], N = 4096
  s   = (Qbd^T k) / max(||k||_head, eps)                    # [12, N] cosine scores
  a   = softmax_N(s)                                        # [12, N]
  out = Wp @ (Wv @ (x_b @ a^T |head-diag) + bv) + bp        # [768]

Key restructurings vs. the reference:
  * V projection commutes with the attention pooling (one query token per
    head): pool x with the attention weights first, then project the pooled
    [C] vector. Only the K projection runs over all N tokens.
  * The dominant GEMMs (K projection, fused score projection, per-head
    norm sums) run in fp8-e4m3 DoubleRow mode (2 contraction rows/cycle).
    Wk is host-scaled by 32 so its entries sit mid-range of e4m3; the
    cosine is scale-invariant so no descaling is needed (the norm path
    divides the 32 back out).
  * x arrives from the host twice: channel-major fp8 (for K/scores) and
    token-major bf16 (for the attention pooling, which needs the higher
    precision since the pooled signal is ~1/sqrt(N) of x's scale). No
    on-device transposes of x, no f32 input traffic, no DRAM bounce.
  * The token-major copy carries a constant-1.0 column so the pooling
    matmul also produces the softmax normalizer (sum of exp) for free.
  * Pooling accumulates into persistent PSUM banks chunk-by-chunk during
    pass A (normalized by the softmax sum at batch end), so the PE never
    drains between the score pass and the pooling pass.
  * DMA queues are dedicated: sync carries wk8 + the x8 f-tile stream +
    attn transposes, gpsimd (SWDGE) carries the big token-major chunks,
    scalar carries the first x8 tile + the remaining weights. Big chunk
    transfers never queue ahead of latency-critical small ones.
  * Batch 0's tail (pooledT transpose, Wv GEMM, head-diagonal select)
    runs during batch 1's main pass; only batch 1's tail + the final Wp
    GEMM are serial.

Distribution: pure data-parallel over batch, 2 batches per core, 8 cores.
No collectives; host scatters inputs / gathers outputs.

Self-contained: hardcodes all shapes; no sibling imports.
"""

import numpy as np
import ml_dtypes

import concourse.bass as bass
import concourse.mybir as mybir
import concourse.tile as tile
from concourse import bacc
from concourse.bass import ts
from concourse.bass_utils import run_bass_kernel_spmd
from concourse.masks import make_identity

F32 = mybir.dt.float32
BF16 = mybir.dt.bfloat16
FP8 = mybir.dt.float8e4
AF = mybir.ActivationFunctionType
OP = mybir.AluOpType
DR = mybir.MatmulPerfMode.DoubleRow

B, C, HEADS, HD = 16, 768, 12, 64
N = 64 * 64                 # tokens per batch
NCORES = 8
BPC = B // NCORES           # batches per core = 2
CT = C // 128               # 6 c-tiles (contraction / channel tiles)
CP = CT // 2                # 3 DoubleRow c-tile pairs
FT = 512                    # token f-tile size
NFT = N // FT               # 8 f-tiles
NCH = 4                     # attn/xT chunks per batch
CHW = N // NCH              # chunk width in tokens (1024)
NTC = CHW // 128            # 128-token tiles per chunk (8)
CS = C + 1                  # xT columns: 768 channels + a ones column
EPS = 1e-12
WKS = 32.0                  # host-side Wk scale (fp8 dynamic-range centering)


def _act_table_filter():
    """Restrict activation-table choice to the single set that covers all
    funcs this kernel uses (Copy/Exp/Ln/Square), so no mid-kernel
    ACT_TABLE_LOAD swaps are emitted. Index positions are preserved."""
    import functools
    import concourse.bacc as _bacc

    orig = _bacc.get_activation_tables

    @functools.cache
    def filtered(arch):
        t = orig(arch)
        return {
            name: (s if name == "natural_log_exp_and_others" else set())
            for name, s in t.items()
        }

    return orig, filtered


def _build_nc():
    nc = bacc.Bacc(
        "TRN2",
        target_bir_lowering=False,
        debug=False,
        enable_asserts=False,
        num_devices=NCORES,
    )

    x8_d = nc.dram_tensor("x8", [BPC, NFT, 128, CP, FT * 2], FP8, kind="ExternalInput").ap()
    xt_d = nc.dram_tensor("xt", [BPC, NCH, 128, NTC, CS], BF16, kind="ExternalInput").ap()
    wk8_d = nc.dram_tensor("wk8", [128, CT, C], FP8, kind="ExternalInput").ap()
    wk2_d = nc.dram_tensor("wk2", [128, CT, C], BF16, kind="ExternalInput").ap()
    wq_d = nc.dram_tensor("wqT", [128, CT, C], BF16, kind="ExternalInput").ap()
    wv_d = nc.dram_tensor("wvT", [128, CT, C], BF16, kind="ExternalInput").ap()
    wp_d = nc.dram_tensor("wpT", [128, CT, C], BF16, kind="ExternalInput").ap()
    aux_d = nc.dram_tensor("aux", [128, CT, 8], F32, kind="ExternalInput").ap()
    z_d = nc.dram_tensor("z", [C, BPC], F32, kind="ExternalOutput").ap()

    with tile.TileContext(nc) as tc:
        _emit(nc, tc, x8_d, xt_d, wk8_d, wk2_d, wq_d, wv_d, wp_d, aux_d, z_d)
    import concourse.bacc as _bacc
    orig, filtered = _act_table_filter()
    _bacc.get_activation_tables = filtered
    try:
        nc.compile()
    finally:
        _bacc.get_activation_tables = orig
    return nc


def _emit(nc, tc, x8_d, xt_d, wk8_d, wk2_d, wq_d, wv_d, wp_d, aux_d, z_d):
    from contextlib import ExitStack

    ctx = ExitStack()
    with ctx:
        const = ctx.enter_context(tc.tile_pool(name="const", bufs=1))
        x8_pool = ctx.enter_context(tc.tile_pool(name="x8", bufs=3))
        k2_pool = ctx.enter_context(tc.tile_pool(name="k2", bufs=3))
        xt_pool = ctx.enter_context(tc.tile_pool(name="xt", bufs=3))
        small = ctx.enter_context(tc.tile_pool(name="small", bufs=4))
        at_pool = ctx.enter_context(tc.tile_pool(name="at", bufs=5))
        kp_pool = ctx.enter_context(tc.tile_pool(name="kp", bufs=4, space="PSUM"))
        sp_pool = ctx.enter_context(tc.tile_pool(name="sp", bufs=1, space="PSUM"))
        sq_pool = ctx.enter_context(tc.tile_pool(name="sq", bufs=1, space="PSUM"))
        pp_pool = ctx.enter_context(tc.tile_pool(name="pp", bufs=2, space="PSUM"))

        # ---- startup DMAs: wk8 leads the sync queue, x8 tile 0 leads the
        # scalar queue, so the first K-proj matmul group has both operands
        # as early as possible.  All other x8 tiles ride sync; the big
        # token-major chunks ride gpsimd/SWDGE exclusively.
        wk8_sb = const.tile([128, CT, C], FP8)
        nc.sync.dma_start(wk8_sb[:, 0:2, :], wk8_d[:, 0:2, :])
        nc.sync.dma_start(wk8_sb[:, 2:6, :], wk8_d[:, 2:6, :])
        x8_first = x8_pool.tile([128, CP, FT * 2], FP8, name="x8_0_0", tag="x8")
        nc.scalar.dma_start(x8_first[:, 0:1, :], x8_d[0, 0, :, 0:1, :])
        nc.scalar.dma_start(x8_first[:, 1:3, :], x8_d[0, 0, :, 1:3, :])
        aux_sb = const.tile([128, CT, 8], F32)
        nc.scalar.dma_start(aux_sb, aux_d)
        wq_sb = const.tile([128, CT, C], BF16)
        nc.scalar.dma_start(wq_sb, wq_d)
        wk2_sb = const.tile([128, CT, C], BF16)
        nc.scalar.dma_start(wk2_sb, wk2_d)
        y_sb = aux_sb[:, :, 4:6]

        id128_bf = const.tile([128, 128], BF16)
        make_identity(nc, id128_bf)
        id32_f = const.tile([32, 32], F32)
        make_identity(nc, id32_f)

        # head indicator masks: mask32_f[c, h] = 1 if channel c belongs to
        # head h (columns padded to 32 for fp8-dual ldweights alignment)
        ones8 = const.tile([128, CT, 32], FP8)
        mask32_f = const.tile([128, CT, 32], F32)
        ones_bf = const.tile([128, CT, HEADS], BF16)
        onesT_bf = const.tile([HEADS, C], BF16)
        nc.vector.memset(ones8, 0.0)
        nc.vector.memset(mask32_f, 0.0)
        nc.vector.memset(ones_bf, 0.0)
        for c in range(CT):
            for half in range(2):
                h = 2 * c + half
                rows = slice(64 * half, 64 * (half + 1))
                nc.vector.memset(ones8[rows, c, h : h + 1], 1.0)
                nc.vector.memset(mask32_f[rows, c, h : h + 1], 1.0)
                nc.vector.memset(ones_bf[rows, c, h : h + 1], 1.0)

        # ---- statics --------------------------------------------------------
        scores_ch = {}
        pooledT_b = [const.tile([32, C], F32, name=f"pooledT{b}") for b in range(BPC)]
        pooled_sb = const.tile([128, CT, BPC * HEADS], BF16)
        outv_sb = const.tile([128, CT, BPC], BF16)

        wtld8 = const.tile([128, CT, 32 * BPC], FP8)
        qbk_sb = const.tile([32 * BPC, 1], F32)

        def qpath():
            y_bf = const.tile([128, CT, BPC], BF16)
            nc.vector.tensor_copy(out=y_bf, in_=y_sb)
            for c in range(CT):
                otp = pp_pool.tile([HEADS, 128], BF16, tag="pp")
                nc.tensor.transpose(otp, ones_bf[:, c, :], id128_bf)
                nc.vector.tensor_copy(out=onesT_bf[:, ts(c, 128)], in_=otp)
            q_sb = const.tile([128, CT, BPC], F32)
            for o in range(CT):
                qp = kp_pool.tile([128, BPC], F32, tag="kp")
                for c in range(CT):
                    nc.tensor.matmul(
                        qp, wq_sb[:, c, ts(o, 128)], y_bf[:, c, :],
                        start=(c == 0), stop=(c == CT - 1),
                    )
                nc.vector.tensor_tensor(
                    out=q_sb[:, o, :], in0=qp,
                    in1=aux_sb[:, o, 0:1].to_broadcast((128, BPC)), op=OP.add,
                )
            q2_sb = const.tile([128, CT, BPC], F32)
            nc.vector.tensor_tensor(out=q2_sb, in0=q_sb, in1=q_sb, op=OP.mult)
            ssqq = kp_pool.tile([HEADS, BPC], F32, tag="kp")
            for c in range(CT):
                nc.tensor.matmul(
                    ssqq, mask32_f[:, c, 0:HEADS], q2_sb[:, c, :],
                    start=(c == 0), stop=(c == CT - 1),
                )
            rq = const.tile([HEADS, BPC], F32)
            nc.scalar.activation(out=rq, in_=ssqq, func=AF.Ln)
            nc.scalar.activation(out=rq, in_=rq, func=AF.Exp, scale=-0.5)
            nc.vector.tensor_scalar_min(rq, rq, 1.0 / EPS)
            rq_bf = const.tile([HEADS, BPC], BF16)
            nc.vector.tensor_copy(out=rq_bf, in_=rq)
            rqbc = kp_pool.tile([128, CT, BPC], F32, tag="kp")
            for c in range(CT):
                nc.tensor.matmul(
                    rqbc[:, c, :], onesT_bf[:, ts(c, 128)], rq_bf,
                    start=(c == 0), stop=(c == CT - 1), skip_group_check=True,
                )
            qn_sb = const.tile([128, CT, BPC], F32)
            nc.vector.tensor_tensor(out=qn_sb, in0=q_sb, in1=rqbc, op=OP.mult)
            # block-diagonal placement: qbd[:, :, 32b+h] = qn[:, :, b] * mask_h
            qbd_f = const.tile([128, CT, 32 * BPC], F32)
            for b in range(BPC):
                nc.vector.tensor_tensor(
                    out=qbd_f[:, :, 32 * b : 32 * b + 32],
                    in0=qn_sb[:, :, b : b + 1].to_broadcast((128, CT, 32)),
                    in1=mask32_f, op=OP.mult,
                )
            qbd_bf = const.tile([128, CT, 32 * BPC], BF16)
            nc.vector.tensor_copy(out=qbd_bf, in_=qbd_f)
            # fold q into the K projection: raw = ((32Wk)^T Qbd)^T x + Qbd^T (32bk)
            for m in range(CT):
                wtp = kp_pool.tile([128, 32 * BPC], F32, tag="kp")
                for ot in range(CT):
                    nc.tensor.matmul(
                        wtp, wk2_sb[:, ot, ts(m, 128)], qbd_bf[:, ot, :],
                        start=(ot == 0), stop=(ot == CT - 1),
                    )
                nc.vector.tensor_copy(out=wtld8[:, m, :], in_=wtp)
            qbkp = kp_pool.tile([32 * BPC, 1], F32, tag="kp")
            for ot in range(CT):
                nc.tensor.matmul(
                    qbkp, qbd_f[:, ot, :], aux_sb[:, ot, 1:2],
                    start=(ot == 0), stop=(ot == CT - 1),
                )
            nc.vector.tensor_copy(out=qbk_sb, in_=qbkp)

        # ---- per-batch pass A ----------------------------------------------
        x8_t = {(0, 0): x8_first}
        k2_t = {}
        xt_t = {}
        att_t = {}
        rse_b = [None] * BPC
        pp_b = [None] * BPC

        def kpart(b, i):
            if (b, i) not in x8_t:
                x8 = x8_pool.tile([128, CP, FT * 2], FP8, name=f"x8_{b}_{i}", tag="x8")
                nc.sync.dma_start(x8, x8_d[b, i])
                x8_t[(b, i)] = x8
            x8 = x8_t[(b, i)]
            x8r = x8.rearrange("p j (f two) -> p j two f", two=2)
            # stage the token-major chunk for pooling on the SWDGE ring
            if i == 0 or i % 2 == 1:
                ch = 0 if i == 0 else (i + 1) // 2
                if ch < NCH:
                    xtc = xt_pool.tile([128, NTC, CS], BF16,
                                       name=f"xt{b}_{ch}", tag="xt")
                    nc.gpsimd.dma_start(xtc, xt_d[b, ch])
                    xt_t[(b, ch)] = xtc
            k2sb = k2_pool.tile([128, CP, FT * 2], FP8, name=f"k2_{b}_{i}", tag="k2")
            k2r = k2sb.rearrange("p j (f two) -> p j two f", two=2)
            k2_t[(b, i)] = k2sb
            for o in range(CT):
                kp = kp_pool.tile([128, FT], F32, tag="kp")
                for j in range(CP):
                    nc.tensor.matmul(
                        kp, wk8_sb[:, 2 * j : 2 * j + 2, ts(o, 128)],
                        x8r[:, j],
                        start=(j == 0), stop=(j == CP - 1), perf_mode=DR,
                    )
                # k2 = (kp/32 + bk)^2, in e4m3 (values <= ~8), written
                # pair-interleaved so the norm-sum matmul streams at full
                # dual-fp8 rate
                nc.scalar.activation(
                    out=k2r[:, o // 2, o % 2, :], in_=kp, func=AF.Square,
                    bias=aux_sb[:, o, 3:4], scale=1.0 / WKS,
                )

        def spart(b, i):
            R = slice(32 * b, 32 * b + HEADS)
            x8r = x8_t.pop((b, i)).rearrange("p j (f two) -> p j two f", two=2)
            k2r = k2_t.pop((b, i)).rearrange("p j (f two) -> p j two f", two=2)
            sp = sp_pool.tile([32 * BPC, FT], F32, tag="sp")
            for j in range(CP):
                nc.tensor.matmul(
                    sp, wtld8[:, 2 * j : 2 * j + 2, :],
                    x8r[:, j],
                    start=(j == 0), stop=(j == CP - 1), perf_mode=DR,
                )
            sq = sq_pool.tile([32, FT], F32, tag="sq")
            for j in range(CP):
                nc.tensor.matmul(
                    sq, ones8[:, 2 * j : 2 * j + 2, :],
                    k2r[:, j],
                    start=(j == 0), stop=(j == CP - 1), perf_mode=DR,
                )
            rt = small.tile([HEADS, FT], F32, tag="rt", bufs=3)
            # rt = (WKS^2 * ssq)^-0.5 = 1/(WKS*||k||); cancels sp's WKS scale
            nc.scalar.activation(out=rt, in_=sq[0:HEADS, :], func=AF.Ln,
                                 scale=WKS * WKS)
            nc.scalar.activation(out=rt, in_=rt, func=AF.Exp, scale=-0.5)
            nc.vector.tensor_scalar_min(rt, rt, 1.0 / EPS)
            nc.vector.tensor_scalar(
                out=sp[R, :], in0=sp[R, :],
                scalar1=qbk_sb[R], scalar2=None, op0=OP.add,
            )
            scores_ch[(b, i)] = small.tile(
                [44, FT], F32, tag="sch", name=f"sch{b}_{i}", bufs=4)
            nc.vector.tensor_tensor(
                out=scores_ch[(b, i)][R, :],
                in0=sp[R, :], in1=rt, op=OP.mult,
            )

        neg1 = const.tile([64, 1], F32)
        nc.vector.memset(neg1, -1.0)

        def exp_ft(b, i):
            # scores are cosines in [-1, 1]: exp(s - 1) is stable without a
            # running max, so the softmax pipeline runs inside pass A.  The
            # softmax normalizer falls out of the pooling matmul (ones col).
            R = slice(32 * b, 32 * b + HEADS)
            abt = at_pool.tile(
                [64, FT], BF16, tag="ab", name=f"ab{b}_{i}", bufs=4)
            nc.scalar.activation(
                out=abt[R, :], in_=scores_ch.pop((b, i))[R, :], func=AF.Exp,
                bias=neg1[R], scale=1.0,
            )
            att = at_pool.tile(
                [128, FT // 128, 32], BF16, tag="attnT", name=f"att{b}_{i}",
                bufs=4)
            nc.sync.dma_start_transpose(att, abt[32 * b : 32 * b + 32, :])
            att_t[(b, i)] = att

        def pool_ft(b, i):
            att = att_t.pop((b, i))
            xtc = xt_t[(b, i // 2)]
            if i % 2 == 1:
                xt_t.pop((b, i // 2))
            if i == 0:
                pp0 = pp_pool.tile([HEADS, 384], F32, tag="pp", name=f"pp0_{b}")
                pp1 = pp_pool.tile([HEADS, 385], F32, tag="pp", name=f"pp1_{b}")
                pp_b[b] = (pp0, pp1)
            pp0, pp1 = pp_b[b]
            for t in range(FT // 128):
                nt = (i % 2) * (FT // 128) + t
                atl = att[:, t, 0:HEADS]
                first = i == 0 and t == 0
                last = i == NFT - 1 and t == FT // 128 - 1
                nc.tensor.matmul(
                    pp0, atl, xtc[:, nt, 0:384],
                    start=first, stop=last, skip_group_check=True,
                )
                nc.tensor.matmul(
                    pp1, atl, xtc[:, nt, 384:769],
                    start=first, stop=last, skip_group_check=True,
                )

        def pool_fin(b):
            # pp1's last column is sum(exp); normalize both pooled halves
            rse = small.tile([HEADS, 1], F32, tag="st", name=f"rse{b}")
            pp0, pp1 = pp_b[b]
            nc.vector.reciprocal(rse, pp1[:, 384:385])
            rse_b[b] = rse
            nc.vector.tensor_scalar_mul(pooledT_b[b][0:HEADS, 0:384], pp0, rse)
            nc.vector.tensor_scalar_mul(
                pooledT_b[b][0:HEADS, 384:768], pp1[:, 0:384], rse)

        wv_sb = const.tile([128, CT, C], BF16)
        wp_sb = const.tile([128, CT, C], BF16)

        def tail_b(b):
            # pooledT[12, C] -> channel-major pooled_sb columns for batch b,
            # then the Wv GEMM + head-diagonal selection for this batch.
            for c in range(CT):
                tpp = kp_pool.tile([128, 32], F32, tag="kp", name=f"tp{b}_{c}")
                nc.tensor.transpose(tpp, pooledT_b[b][:, ts(c, 128)], id32_f)
                nc.vector.tensor_copy(
                    out=pooled_sb[:, c, b * HEADS : (b + 1) * HEADS],
                    in_=tpp[:, 0:HEADS])
            for o in range(CT):
                vp = kp_pool.tile([128, HEADS], F32, tag="kp", name=f"vp{b}_{o}")
                for c in range(CT):
                    nc.tensor.matmul(
                        vp, wv_sb[:, c, ts(o, 128)],
                        pooled_sb[:, c, b * HEADS : (b + 1) * HEADS],
                        start=(c == 0), stop=(c == CT - 1),
                    )
                for half in range(2):
                    h = 2 * o + half
                    rows = slice(64 * half, 64 * (half + 1))
                    nc.vector.tensor_copy(
                        out=outv_sb[rows, o, b : b + 1],
                        in_=vp[rows, h : h + 1],
                    )

        # ---- schedule -------------------------------------------------------
        kpart(0, 0)
        kpart(0, 1)
        kpart(0, 2)
        qpath()
        for i in range(NFT):
            spart(0, i)
            exp_ft(0, i)
            if i >= 1:
                pool_ft(0, i - 1)
            if i + 3 < NFT:
                kpart(0, i + 3)
        kpart(1, 0)
        nc.scalar.dma_start(wv_sb, wv_d)
        nc.scalar.dma_start(wp_sb, wp_d)
        pool_ft(0, NFT - 1)
        pool_fin(0)
        kpart(1, 1)
        kpart(1, 2)
        tail_b(0)
        for i in range(NFT):
            spart(1, i)
            exp_ft(1, i)
            if i >= 1:
                pool_ft(1, i - 1)
            if i + 3 < NFT:
                kpart(1, i + 3)
        pool_ft(1, NFT - 1)
        pool_fin(1)
        tail_b(1)

        # ---- final Wp GEMM + bias + output ---------------------------------
        z_sb = const.tile([128, CT, BPC], F32)
        for o2 in range(CT):
            zp = kp_pool.tile([128, BPC], F32, tag="kp")
            for o in range(CT):
                nc.tensor.matmul(
                    zp, wp_sb[:, o, ts(o2, 128)], outv_sb[:, o, :],
                    start=(o == 0), stop=(o == CT - 1),
                )
            nc.vector.tensor_tensor(
                out=z_sb[:, o2, :], in0=zp,
                in1=aux_sb[:, o2, 2:3].to_broadcast((128, BPC)), op=OP.add,
            )
        nc.sync.dma_start(z_d.rearrange("(c p) b -> p c b", p=128), z_sb)


_NC_CACHE = None


def _get_nc():
    global _NC_CACHE
    if _NC_CACHE is None:
        _NC_CACHE = _build_nc()
    return _NC_CACHE


def make_in_maps(inputs):
    x = np.ascontiguousarray(np.asarray(inputs["x"], dtype=np.float32)).reshape(B, C, N)
    y = np.asarray(inputs["y"], dtype=np.float32).reshape(B, C)
    Wq = np.asarray(inputs["Wq"], dtype=np.float32)
    bq = np.asarray(inputs["bq"], dtype=np.float32)
    Wkv = np.asarray(inputs["Wkv"], dtype=np.float32)
    bkv = np.asarray(inputs["bkv"], dtype=np.float32)
    Wp = np.asarray(inputs["Wp"], dtype=np.float32)
    bp = np.asarray(inputs["bp"], dtype=np.float32)

    wk, wv = Wkv[:C], Wkv[C:]
    bk, bv = bkv[:C], bkv[C:]

    def ptile(wT, dt=ml_dtypes.bfloat16):
        # [C, C] (contraction-major) -> [128, CT, C] SBUF layout
        return np.ascontiguousarray(
            wT.reshape(CT, 128, C).transpose(1, 0, 2)).astype(dt)

    wk8 = ptile(wk.T * WKS, ml_dtypes.float8_e4m3)
    wk2 = ptile(wk * WKS)
    wqT = ptile(Wq.T)
    wvT = ptile(wv.T)
    wpT = ptile(Wp.T)
    bpz = (Wp @ bv + bp).astype(np.float32)

    # channel-major fp8 x with DoubleRow pair interleave: element order
    # per partition row is (pair j, token f, subtile e): addr = 2f + e
    x8 = np.ascontiguousarray(
        x.reshape(B, CP, 2, 128, NFT, FT).transpose(0, 4, 3, 1, 5, 2)
        .reshape(B, NFT, 128, CP, FT * 2)
    ).astype(ml_dtypes.float8_e4m3)
    # token-major bf16 x + ones col: [B, NCH, 128, NTC, CS]
    xt = np.ones((B, NCH, 128, NTC, CS), ml_dtypes.bfloat16)
    xt[..., :C] = x.reshape(B, C, NCH, NTC, 128).transpose(
        0, 2, 4, 3, 1).astype(ml_dtypes.bfloat16)

    def pcol(v):
        return v.reshape(CT, 128).T  # [(c p)] -> [p, c]

    in_maps = []
    for i in range(NCORES):
        aux = np.zeros((128, CT, 8), np.float32)
        aux[:, :, 0] = pcol(bq)
        aux[:, :, 1] = pcol(bk * WKS)
        aux[:, :, 2] = pcol(bpz)
        aux[:, :, 3] = pcol(bk)
        yb = y[i * BPC : (i + 1) * BPC]  # [2, C]
        for b in range(BPC):
            aux[:, :, 4 + b] = pcol(yb[b])
        in_maps.append({
            "x8": np.ascontiguousarray(x8[i * BPC : (i + 1) * BPC]),
            "xt": np.ascontiguousarray(xt[i * BPC : (i + 1) * BPC]),
            "wk8": wk8, "wk2": wk2, "wqT": wqT, "wvT": wvT, "wpT": wpT,
            "aux": aux,
        })
    return in_maps


def kernel(**inputs):
    nc = _get_nc()
    in_maps = make_in_maps(inputs)
    res = run_bass_kernel_spmd(nc, in_maps, core_ids=list(range(NCORES)))
    z = np.concatenate([r["z"].T for r in res.results], axis=0)
    return z.reshape(B, C, 1, 1).astype(np.float32)


# revision 9
# speedup vs baseline: 1.0226x; 1.0226x over previous
"""Trainium2 Bass kernel for nn_C_Cross_Attention3D (B=16, C=768, H=W=64, HEADS=12).

Math (per batch b):
  q   = l2norm_per_head(Wq @ y_b + bq)                      # [12, 64]
  k   = Wk @ x_b + bk                                       # [768, N], N = 4096
  s   = (Qbd^T k) / max(||k||_head, eps)                    # [12, N] cosine scores
  a   = softmax_N(s)                                        # [12, N]
  out = Wp @ (Wv @ (x_b @ a^T |head-diag) + bv) + bp        # [768]

Key restructurings vs. the reference:
  * V projection commutes with the attention pooling (one query token per
    head): pool x with the attention weights first, then project the pooled
    [C] vector. Only the K projection runs over all N tokens.
  * The dominant GEMMs (K projection, fused score projection, per-head
    norm sums) run in fp8-e4m3 DoubleRow mode (2 contraction rows/cycle).
    Wk is host-scaled by 32 so its entries sit mid-range of e4m3; the
    cosine is scale-invariant so no descaling is needed (the norm path
    divides the 32 back out).
  * x arrives from the host twice: channel-major fp8 (for K/scores) and
    token-major bf16 (for the attention pooling, which needs the higher
    precision since the pooled signal is ~1/sqrt(N) of x's scale). No
    on-device transposes of x, no f32 input traffic, no DRAM bounce.
  * The token-major copy carries a constant-1.0 column so the pooling
    matmul also produces the softmax normalizer (sum of exp) for free.
  * Pooling accumulates into persistent PSUM banks chunk-by-chunk during
    pass A (normalized by the softmax sum at batch end), so the PE never
    drains between the score pass and the pooling pass.
  * DMA queues are dedicated: sync carries wk8 + the x8 f-tile stream +
    attn transposes, gpsimd (SWDGE) carries the big token-major chunks,
    scalar carries the first x8 tile + the remaining weights. Big chunk
    transfers never queue ahead of latency-critical small ones.
  * Batch 0's tail (pooledT transpose, Wv GEMM, head-diagonal select)
    runs during batch 1's main pass; only batch 1's tail + the final Wp
    GEMM are serial.

Distribution: pure data-parallel over batch, 2 batches per core, 8 cores.
No collectives; host scatters inputs / gathers outputs.

Self-contained: hardcodes all shapes; no sibling imports.
"""

import numpy as np
import ml_dtypes

import concourse.bass as bass
import concourse.mybir as mybir
import concourse.tile as tile
from concourse import bacc
from concourse.bass import ts
from concourse.bass_utils import run_bass_kernel_spmd
from concourse.masks import make_identity

F32 = mybir.dt.float32
BF16 = mybir.dt.bfloat16
FP8 = mybir.dt.float8e4
AF = mybir.ActivationFunctionType
OP = mybir.AluOpType
DR = mybir.MatmulPerfMode.DoubleRow

B, C, HEADS, HD = 16, 768, 12, 64
N = 64 * 64                 # tokens per batch
NCORES = 8
BPC = B // NCORES           # batches per core = 2
CT = C // 128               # 6 c-tiles (contraction / channel tiles)
CP = CT // 2                # 3 DoubleRow c-tile pairs
FT = 512                    # token f-tile size
NFT = N // FT               # 8 f-tiles
NCH = 4                     # attn/xT chunks per batch
CHW = N // NCH              # chunk width in tokens (1024)
NTC = CHW // 128            # 128-token tiles per chunk (8)
CS = C + 1                  # xT columns: 768 channels + a ones column
EPS = 1e-12
WKS = 32.0                  # host-side Wk scale (fp8 dynamic-range centering)


def _act_table_filter():
    """Restrict activation-table choice to the single set that covers all
    funcs this kernel uses (Copy/Exp/Ln/Square), so no mid-kernel
    ACT_TABLE_LOAD swaps are emitted. Index positions are preserved."""
    import functools
    import concourse.bacc as _bacc

    orig = _bacc.get_activation_tables

    @functools.cache
    def filtered(arch):
        t = orig(arch)
        return {
            name: (s if name == "natural_log_exp_and_others" else set())
            for name, s in t.items()
        }

    return orig, filtered


def _build_nc():
    nc = bacc.Bacc(
        "TRN2",
        target_bir_lowering=False,
        debug=False,
        enable_asserts=False,
        num_devices=NCORES,
    )

    x8_d = nc.dram_tensor("x8", [BPC, NFT, 128, CT, FT], FP8, kind="ExternalInput").ap()
    xt_d = nc.dram_tensor("xt", [BPC, NCH, 128, NTC, CS], BF16, kind="ExternalInput").ap()
    wk8_d = nc.dram_tensor("wk8", [128, CT, C], FP8, kind="ExternalInput").ap()
    wk2_d = nc.dram_tensor("wk2", [128, CT, C], BF16, kind="ExternalInput").ap()
    wq_d = nc.dram_tensor("wqT", [128, CT, C], BF16, kind="ExternalInput").ap()
    wv_d = nc.dram_tensor("wvT", [128, CT, C], BF16, kind="ExternalInput").ap()
    wp_d = nc.dram_tensor("wpT", [128, CT, C], BF16, kind="ExternalInput").ap()
    aux_d = nc.dram_tensor("aux", [128, CT, 8], F32, kind="ExternalInput").ap()
    z_d = nc.dram_tensor("z", [C, BPC], F32, kind="ExternalOutput").ap()

    with tile.TileContext(nc) as tc:
        _emit(nc, tc, x8_d, xt_d, wk8_d, wk2_d, wq_d, wv_d, wp_d, aux_d, z_d)
    import concourse.bacc as _bacc
    orig, filtered = _act_table_filter()
    _bacc.get_activation_tables = filtered
    try:
        nc.compile()
    finally:
        _bacc.get_activation_tables = orig
    return nc


def _emit(nc, tc, x8_d, xt_d, wk8_d, wk2_d, wq_d, wv_d, wp_d, aux_d, z_d):
    from contextlib import ExitStack

    ctx = ExitStack()
    with ctx:
        const = ctx.enter_context(tc.tile_pool(name="const", bufs=1))
        x8_pool = ctx.enter_context(tc.tile_pool(name="x8", bufs=3))
        k2_pool = ctx.enter_context(tc.tile_pool(name="k2", bufs=3))
        xt_pool = ctx.enter_context(tc.tile_pool(name="xt", bufs=3))
        small = ctx.enter_context(tc.tile_pool(name="small", bufs=4))
        at_pool = ctx.enter_context(tc.tile_pool(name="at", bufs=5))
        kp_pool = ctx.enter_context(tc.tile_pool(name="kp", bufs=4, space="PSUM"))
        sp_pool = ctx.enter_context(tc.tile_pool(name="sp", bufs=1, space="PSUM"))
        sq_pool = ctx.enter_context(tc.tile_pool(name="sq", bufs=1, space="PSUM"))
        pp_pool = ctx.enter_context(tc.tile_pool(name="pp", bufs=2, space="PSUM"))

        # ---- startup DMAs: wk8 leads the sync queue, x8 tile 0 leads the
        # scalar queue, so the first K-proj matmul group has both operands
        # as early as possible.  All other x8 tiles ride sync; the big
        # token-major chunks ride gpsimd/SWDGE exclusively.
        wk8_sb = const.tile([128, CT, C], FP8)
        nc.sync.dma_start(wk8_sb[:, 0:2, :], wk8_d[:, 0:2, :])
        nc.sync.dma_start(wk8_sb[:, 2:6, :], wk8_d[:, 2:6, :])
        x8_first = x8_pool.tile([128, CT, FT], FP8, name="x8_0_0", tag="x8")
        nc.scalar.dma_start(x8_first[:, 0:2, :], x8_d[0, 0, :, 0:2, :])
        nc.scalar.dma_start(x8_first[:, 2:6, :], x8_d[0, 0, :, 2:6, :])
        aux_sb = const.tile([128, CT, 8], F32)
        nc.scalar.dma_start(aux_sb, aux_d)
        wq_sb = const.tile([128, CT, C], BF16)
        nc.scalar.dma_start(wq_sb, wq_d)
        wk2_sb = const.tile([128, CT, C], BF16)
        nc.scalar.dma_start(wk2_sb, wk2_d)
        y_sb = aux_sb[:, :, 4:6]

        id128_bf = const.tile([128, 128], BF16)
        make_identity(nc, id128_bf)
        id32_f = const.tile([32, 32], F32)
        make_identity(nc, id32_f)

        # head indicator masks: mask32_f[c, h] = 1 if channel c belongs to
        # head h (columns padded to 32 for fp8-dual ldweights alignment)
        ones8 = const.tile([128, CT, 32], FP8)
        mask32_f = const.tile([128, CT, 32], F32)
        ones_bf = const.tile([128, CT, HEADS], BF16)
        onesT_bf = const.tile([HEADS, C], BF16)
        nc.vector.memset(ones8, 0.0)
        nc.vector.memset(mask32_f, 0.0)
        nc.vector.memset(ones_bf, 0.0)
        for c in range(CT):
            for half in range(2):
                h = 2 * c + half
                rows = slice(64 * half, 64 * (half + 1))
                nc.vector.memset(ones8[rows, c, h : h + 1], 1.0)
                nc.vector.memset(mask32_f[rows, c, h : h + 1], 1.0)
                nc.vector.memset(ones_bf[rows, c, h : h + 1], 1.0)

        # ---- statics --------------------------------------------------------
        scores_ch = {}
        pooledT_b = [const.tile([32, C], F32, name=f"pooledT{b}") for b in range(BPC)]
        pooled_sb = const.tile([128, CT, BPC * HEADS], BF16)
        outv_sb = const.tile([128, CT, BPC], BF16)

        wtld8 = const.tile([128, CT, 32 * BPC], FP8)
        qbk_sb = const.tile([32 * BPC, 1], F32)

        def qpath():
            y_bf = const.tile([128, CT, BPC], BF16)
            nc.vector.tensor_copy(out=y_bf, in_=y_sb)
            for c in range(CT):
                otp = pp_pool.tile([HEADS, 128], BF16, tag="pp")
                nc.tensor.transpose(otp, ones_bf[:, c, :], id128_bf)
                nc.vector.tensor_copy(out=onesT_bf[:, ts(c, 128)], in_=otp)
            q_sb = const.tile([128, CT, BPC], F32)
            for o in range(CT):
                qp = kp_pool.tile([128, BPC], F32, tag="kp")
                for c in range(CT):
                    nc.tensor.matmul(
                        qp, wq_sb[:, c, ts(o, 128)], y_bf[:, c, :],
                        start=(c == 0), stop=(c == CT - 1),
                    )
                nc.vector.tensor_tensor(
                    out=q_sb[:, o, :], in0=qp,
                    in1=aux_sb[:, o, 0:1].to_broadcast((128, BPC)), op=OP.add,
                )
            q2_sb = const.tile([128, CT, BPC], F32)
            nc.vector.tensor_tensor(out=q2_sb, in0=q_sb, in1=q_sb, op=OP.mult)
            ssqq = kp_pool.tile([HEADS, BPC], F32, tag="kp")
            for c in range(CT):
                nc.tensor.matmul(
                    ssqq, mask32_f[:, c, 0:HEADS], q2_sb[:, c, :],
                    start=(c == 0), stop=(c == CT - 1),
                )
            rq = const.tile([HEADS, BPC], F32)
            nc.scalar.activation(out=rq, in_=ssqq, func=AF.Ln)
            nc.scalar.activation(out=rq, in_=rq, func=AF.Exp, scale=-0.5)
            nc.vector.tensor_scalar_min(rq, rq, 1.0 / EPS)
            rq_bf = const.tile([HEADS, BPC], BF16)
            nc.vector.tensor_copy(out=rq_bf, in_=rq)
            rqbc = kp_pool.tile([128, CT, BPC], F32, tag="kp")
            for c in range(CT):
                nc.tensor.matmul(
                    rqbc[:, c, :], onesT_bf[:, ts(c, 128)], rq_bf,
                    start=(c == 0), stop=(c == CT - 1), skip_group_check=True,
                )
            qn_sb = const.tile([128, CT, BPC], F32)
            nc.vector.tensor_tensor(out=qn_sb, in0=q_sb, in1=rqbc, op=OP.mult)
            # block-diagonal placement: qbd[:, :, 32b+h] = qn[:, :, b] * mask_h
            qbd_f = const.tile([128, CT, 32 * BPC], F32)
            for b in range(BPC):
                nc.vector.tensor_tensor(
                    out=qbd_f[:, :, 32 * b : 32 * b + 32],
                    in0=qn_sb[:, :, b : b + 1].to_broadcast((128, CT, 32)),
                    in1=mask32_f, op=OP.mult,
                )
            qbd_bf = const.tile([128, CT, 32 * BPC], BF16)
            nc.vector.tensor_copy(out=qbd_bf, in_=qbd_f)
            # fold q into the K projection: raw = ((32Wk)^T Qbd)^T x + Qbd^T (32bk)
            for m in range(CT):
                wtp = kp_pool.tile([128, 32 * BPC], F32, tag="kp")
                for ot in range(CT):
                    nc.tensor.matmul(
                        wtp, wk2_sb[:, ot, ts(m, 128)], qbd_bf[:, ot, :],
                        start=(ot == 0), stop=(ot == CT - 1),
                    )
                nc.vector.tensor_copy(out=wtld8[:, m, :], in_=wtp)
            qbkp = kp_pool.tile([32 * BPC, 1], F32, tag="kp")
            for ot in range(CT):
                nc.tensor.matmul(
                    qbkp, qbd_f[:, ot, :], aux_sb[:, ot, 1:2],
                    start=(ot == 0), stop=(ot == CT - 1),
                )
            nc.vector.tensor_copy(out=qbk_sb, in_=qbkp)

        # ---- per-batch pass A ----------------------------------------------
        x8_t = {(0, 0): x8_first}
        k2_t = {}
        xt_t = {}
        att_t = {}
        rse_b = [None] * BPC
        pp_b = [None] * BPC

        def kpart(b, i):
            if (b, i) not in x8_t:
                x8 = x8_pool.tile([128, CT, FT], FP8, name=f"x8_{b}_{i}", tag="x8")
                nc.sync.dma_start(x8, x8_d[b, i])
                x8_t[(b, i)] = x8
            x8 = x8_t[(b, i)]
            # stage the token-major chunk for pooling on the SWDGE ring
            if i == 0 or i % 2 == 1:
                ch = 0 if i == 0 else (i + 1) // 2
                if ch < NCH:
                    xtc = xt_pool.tile([128, NTC, CS], BF16,
                                       name=f"xt{b}_{ch}", tag="xt")
                    nc.gpsimd.dma_start(xtc, xt_d[b, ch])
                    xt_t[(b, ch)] = xtc
            k2sb = k2_pool.tile([128, CT, FT], FP8, name=f"k2_{b}_{i}", tag="k2")
            k2_t[(b, i)] = k2sb
            for o in range(CT):
                kp = kp_pool.tile([128, FT], F32, tag="kp")
                for j in range(CP):
                    nc.tensor.matmul(
                        kp, wk8_sb[:, 2 * j : 2 * j + 2, ts(o, 128)],
                        x8[:, 2 * j : 2 * j + 2, :],
                        start=(j == 0), stop=(j == CP - 1), perf_mode=DR,
                    )
                # k2 = (kp/32 + bk)^2, in e4m3 (values <= ~8)
                nc.scalar.activation(
                    out=k2sb[:, o, :], in_=kp, func=AF.Square,
                    bias=aux_sb[:, o, 3:4], scale=1.0 / WKS,
                )

        def spart(b, i):
            R = slice(32 * b, 32 * b + HEADS)
            x8 = x8_t.pop((b, i))
            k2sb = k2_t.pop((b, i))
            sp = sp_pool.tile([32 * BPC, FT], F32, tag="sp")
            for j in range(CP):
                nc.tensor.matmul(
                    sp, wtld8[:, 2 * j : 2 * j + 2, :],
                    x8[:, 2 * j : 2 * j + 2, :],
                    start=(j == 0), stop=(j == CP - 1), perf_mode=DR,
                )
            sq = sq_pool.tile([32, FT], F32, tag="sq")
            for j in range(CP):
                nc.tensor.matmul(
                    sq, ones8[:, 2 * j : 2 * j + 2, :],
                    k2sb[:, 2 * j : 2 * j + 2, :],
                    start=(j == 0), stop=(j == CP - 1), perf_mode=DR,
                )
            rt = small.tile([HEADS, FT], F32, tag="rt", bufs=3)
            # rt = (WKS^2 * ssq)^-0.5 = 1/(WKS*||k||); cancels sp's WKS scale
            nc.scalar.activation(out=rt, in_=sq[0:HEADS, :], func=AF.Ln,
                                 scale=WKS * WKS)
            nc.scalar.activation(out=rt, in_=rt, func=AF.Exp, scale=-0.5)
            nc.vector.tensor_scalar_min(rt, rt, 1.0 / EPS)
            nc.vector.tensor_scalar(
                out=sp[R, :], in0=sp[R, :],
                scalar1=qbk_sb[R], scalar2=None, op0=OP.add,
            )
            scores_ch[(b, i)] = small.tile(
                [44, FT], F32, tag="sch", name=f"sch{b}_{i}", bufs=4)
            nc.vector.tensor_tensor(
                out=scores_ch[(b, i)][R, :],
                in0=sp[R, :], in1=rt, op=OP.mult,
            )

        neg1 = const.tile([64, 1], F32)
        nc.vector.memset(neg1, -1.0)

        def exp_ft(b, i):
            # scores are cosines in [-1, 1]: exp(s - 1) is stable without a
            # running max, so the softmax pipeline runs inside pass A.  The
            # softmax normalizer falls out of the pooling matmul (ones col).
            R = slice(32 * b, 32 * b + HEADS)
            abt = at_pool.tile(
                [64, FT], BF16, tag="ab", name=f"ab{b}_{i}", bufs=4)
            nc.scalar.activation(
                out=abt[R, :], in_=scores_ch.pop((b, i))[R, :], func=AF.Exp,
                bias=neg1[R], scale=1.0,
            )
            att = at_pool.tile(
                [128, FT // 128, 32], BF16, tag="attnT", name=f"att{b}_{i}",
                bufs=4)
            nc.sync.dma_start_transpose(att, abt[32 * b : 32 * b + 32, :])
            att_t[(b, i)] = att

        def pool_ft(b, i):
            att = att_t.pop((b, i))
            xtc = xt_t[(b, i // 2)]
            if i % 2 == 1:
                xt_t.pop((b, i // 2))
            if i == 0:
                pp0 = pp_pool.tile([HEADS, 384], F32, tag="pp", name=f"pp0_{b}")
                pp1 = pp_pool.tile([HEADS, 385], F32, tag="pp", name=f"pp1_{b}")
                pp_b[b] = (pp0, pp1)
            pp0, pp1 = pp_b[b]
            for t in range(FT // 128):
                nt = (i % 2) * (FT // 128) + t
                atl = att[:, t, 0:HEADS]
                first = i == 0 and t == 0
                last = i == NFT - 1 and t == FT // 128 - 1
                nc.tensor.matmul(
                    pp0, atl, xtc[:, nt, 0:384],
                    start=first, stop=last, skip_group_check=True,
                )
                nc.tensor.matmul(
                    pp1, atl, xtc[:, nt, 384:769],
                    start=first, stop=last, skip_group_check=True,
                )

        def pool_fin(b):
            # pp1's last column is sum(exp); normalize both pooled halves
            rse = small.tile([HEADS, 1], F32, tag="st", name=f"rse{b}")
            pp0, pp1 = pp_b[b]
            nc.vector.reciprocal(rse, pp1[:, 384:385])
            rse_b[b] = rse
            nc.vector.tensor_scalar_mul(pooledT_b[b][0:HEADS, 0:384], pp0, rse)
            nc.vector.tensor_scalar_mul(
                pooledT_b[b][0:HEADS, 384:768], pp1[:, 0:384], rse)

        wv_sb = const.tile([128, CT, C], BF16)
        wp_sb = const.tile([128, CT, C], BF16)

        def tail_b(b):
            # pooledT[12, C] -> channel-major pooled_sb columns for batch b,
            # then the Wv GEMM + head-diagonal selection for this batch.
            for c in range(CT):
                tpp = kp_pool.tile([128, 32], F32, tag="kp", name=f"tp{b}_{c}")
                nc.tensor.transpose(tpp, pooledT_b[b][:, ts(c, 128)], id32_f)
                nc.vector.tensor_copy(
                    out=pooled_sb[:, c, b * HEADS : (b + 1) * HEADS],
                    in_=tpp[:, 0:HEADS])
            for o in range(CT):
                vp = kp_pool.tile([128, HEADS], F32, tag="kp", name=f"vp{b}_{o}")
                for c in range(CT):
                    nc.tensor.matmul(
                        vp, wv_sb[:, c, ts(o, 128)],
                        pooled_sb[:, c, b * HEADS : (b + 1) * HEADS],
                        start=(c == 0), stop=(c == CT - 1),
                    )
                for half in range(2):
                    h = 2 * o + half
                    rows = slice(64 * half, 64 * (half + 1))
                    nc.vector.tensor_copy(
                        out=outv_sb[rows, o, b : b + 1],
                        in_=vp[rows, h : h + 1],
                    )

        # ---- schedule -------------------------------------------------------
        kpart(0, 0)
        kpart(0, 1)
        kpart(0, 2)
        qpath()
        for i in range(NFT):
            spart(0, i)
            exp_ft(0, i)
            if i >= 1:
                pool_ft(0, i - 1)
            if i + 3 < NFT:
                kpart(0, i + 3)
        kpart(1, 0)
        nc.scalar.dma_start(wv_sb, wv_d)
        nc.scalar.dma_start(wp_sb, wp_d)
        pool_ft(0, NFT - 1)
        pool_fin(0)
        kpart(1, 1)
        kpart(1, 2)
        tail_b(0)
        for i in range(NFT):
            spart(1, i)
            exp_ft(1, i)
            if i >= 1:
                pool_ft(1, i - 1)
            if i + 3 < NFT:
                kpart(1, i + 3)
        pool_ft(1, NFT - 1)
        pool_fin(1)
        tail_b(1)

        # ---- final Wp GEMM + bias + output ---------------------------------
        z_sb = const.tile([128, CT, BPC], F32)
        for o2 in range(CT):
            zp = kp_pool.tile([128, BPC], F32, tag="kp")
            for o in range(CT):
                nc.tensor.matmul(
                    zp, wp_sb[:, o, ts(o2, 128)], outv_sb[:, o, :],
                    start=(o == 0), stop=(o == CT - 1),
                )
            nc.vector.tensor_tensor(
                out=z_sb[:, o2, :], in0=zp,
                in1=aux_sb[:, o2, 2:3].to_broadcast((128, BPC)), op=OP.add,
            )
        nc.sync.dma_start(z_d.rearrange("(c p) b -> p c b", p=128), z_sb)


_NC_CACHE = None


def _get_nc():
    global _NC_CACHE
    if _NC_CACHE is None:
        _NC_CACHE = _build_nc()
    return _NC_CACHE


def make_in_maps(inputs):
    x = np.ascontiguousarray(np.asarray(inputs["x"], dtype=np.float32)).reshape(B, C, N)
    y = np.asarray(inputs["y"], dtype=np.float32).reshape(B, C)
    Wq = np.asarray(inputs["Wq"], dtype=np.float32)
    bq = np.asarray(inputs["bq"], dtype=np.float32)
    Wkv = np.asarray(inputs["Wkv"], dtype=np.float32)
    bkv = np.asarray(inputs["bkv"], dtype=np.float32)
    Wp = np.asarray(inputs["Wp"], dtype=np.float32)
    bp = np.asarray(inputs["bp"], dtype=np.float32)

    wk, wv = Wkv[:C], Wkv[C:]
    bk, bv = bkv[:C], bkv[C:]

    def ptile(wT, dt=ml_dtypes.bfloat16):
        # [C, C] (contraction-major) -> [128, CT, C] SBUF layout
        return np.ascontiguousarray(
            wT.reshape(CT, 128, C).transpose(1, 0, 2)).astype(dt)

    wk8 = ptile(wk.T * WKS, ml_dtypes.float8_e4m3)
    wk2 = ptile(wk * WKS)
    wqT = ptile(Wq.T)
    wvT = ptile(wv.T)
    wpT = ptile(Wp.T)
    bpz = (Wp @ bv + bp).astype(np.float32)

    # channel-major fp8 x: [B, NFT, 128, CT, FT], channel = ct*128 + p
    x8 = np.ascontiguousarray(
        x.reshape(B, CT, 128, NFT, FT).transpose(0, 3, 2, 1, 4)
    ).astype(ml_dtypes.float8_e4m3)
    # token-major bf16 x + ones col: [B, NCH, 128, NTC, CS]
    xt = np.ones((B, NCH, 128, NTC, CS), ml_dtypes.bfloat16)
    xt[..., :C] = x.reshape(B, C, NCH, NTC, 128).transpose(
        0, 2, 4, 3, 1).astype(ml_dtypes.bfloat16)

    def pcol(v):
        return v.reshape(CT, 128).T  # [(c p)] -> [p, c]

    in_maps = []
    for i in range(NCORES):
        aux = np.zeros((128, CT, 8), np.float32)
        aux[:, :, 0] = pcol(bq)
        aux[:, :, 1] = pcol(bk * WKS)
        aux[:, :, 2] = pcol(bpz)
        aux[:, :, 3] = pcol(bk)
        yb = y[i * BPC : (i + 1) * BPC]  # [2, C]
        for b in range(BPC):
            aux[:, :, 4 + b] = pcol(yb[b])
        in_maps.append({
            "x8": np.ascontiguousarray(x8[i * BPC : (i + 1) * BPC]),
            "xt": np.ascontiguousarray(xt[i * BPC : (i + 1) * BPC]),
            "wk8": wk8, "wk2": wk2, "wqT": wqT, "wvT": wvT, "wpT": wpT,
            "aux": aux,
        })
    return in_maps


def kernel(**inputs):
    nc = _get_nc()
    in_maps = make_in_maps(inputs)
    res = run_bass_kernel_spmd(nc, in_maps, core_ids=list(range(NCORES)))
    z = np.concatenate([r["z"].T for r in res.results], axis=0)
    return z.reshape(B, C, 1, 1).astype(np.float32)


# revision 11
# speedup vs baseline: 1.0315x; 1.0087x over previous
"""Trainium2 Bass kernel for nn_C_Cross_Attention3D (B=16, C=768, H=W=64, HEADS=12).

Math (per batch b):
  q   = l2norm_per_head(Wq @ y_b + bq)                      # [12, 64]
  k   = Wk @ x_b + bk                                       # [768, N], N = 4096
  s   = (Qbd^T k) / max(||k||_head, eps)                    # [12, N] cosine scores
  a   = softmax_N(s)                                        # [12, N]
  out = Wp @ (Wv @ (x_b @ a^T |head-diag) + bv) + bp        # [768]

Key restructurings vs. the reference:
  * V projection commutes with the attention pooling (one query token per
    head): pool x with the attention weights first, then project the pooled
    [C] vector. Only the K projection runs over all N tokens.
  * The dominant GEMMs (K projection, fused score projection, per-head
    norm sums) run in fp8-e4m3 DoubleRow mode (2 contraction rows/cycle).
    Wk is host-scaled by 32 so its entries sit mid-range of e4m3; the
    cosine is scale-invariant so no descaling is needed (the norm path
    divides the 32 back out).
  * x arrives from the host twice: channel-major fp8 (for K/scores) and
    token-major bf16 (for the attention pooling, which needs the higher
    precision since the pooled signal is ~1/sqrt(N) of x's scale). No
    on-device transposes of x, no f32 input traffic, no DRAM bounce.
  * The token-major copy carries a constant-1.0 column so the pooling
    matmul also produces the softmax normalizer (sum of exp) for free.
  * Pooling accumulates into persistent PSUM banks chunk-by-chunk during
    pass A (normalized by the softmax sum at batch end), so the PE never
    drains between the score pass and the pooling pass.
  * DMA queues are dedicated: sync carries wk8 + the x8 f-tile stream +
    attn transposes, gpsimd (SWDGE) carries the big token-major chunks,
    scalar carries the first x8 tile + the remaining weights. Big chunk
    transfers never queue ahead of latency-critical small ones.
  * Batch 0's tail (pooledT transpose, Wv GEMM, head-diagonal select)
    runs during batch 1's main pass; only batch 1's tail + the final Wp
    GEMM are serial.

Distribution: pure data-parallel over batch, 2 batches per core, 8 cores.
No collectives; host scatters inputs / gathers outputs.

Self-contained: hardcodes all shapes; no sibling imports.
"""

import numpy as np
import ml_dtypes

import concourse.bass as bass
import concourse.mybir as mybir
import concourse.tile as tile
from concourse import bacc
from concourse.bass import ts
from concourse.bass_utils import run_bass_kernel_spmd
from concourse.masks import make_identity

F32 = mybir.dt.float32
BF16 = mybir.dt.bfloat16
FP8 = mybir.dt.float8e4
AF = mybir.ActivationFunctionType
OP = mybir.AluOpType
DR = mybir.MatmulPerfMode.DoubleRow

B, C, HEADS, HD = 16, 768, 12, 64
N = 64 * 64                 # tokens per batch
NCORES = 8
BPC = B // NCORES           # batches per core = 2
CT = C // 128               # 6 c-tiles (contraction / channel tiles)
CP = CT // 2                # 3 DoubleRow c-tile pairs
FT = 512                    # token f-tile size
NFT = N // FT               # 8 f-tiles
NCH = 4                     # attn/xT chunks per batch
CHW = N // NCH              # chunk width in tokens (1024)
NTC = CHW // 128            # 128-token tiles per chunk (8)
CS = C + 1                  # xT columns: 768 channels + a ones column
EPS = 1e-12
WKS = 32.0
HAS_BK = False             # set by kernel() from the actual bkv input                  # host-side Wk scale (fp8 dynamic-range centering)


def _act_table_filter():
    """Restrict activation-table choice to the single set that covers all
    funcs this kernel uses (Copy/Exp/Ln/Square), so no mid-kernel
    ACT_TABLE_LOAD swaps are emitted. Index positions are preserved."""
    import functools
    import concourse.bacc as _bacc

    orig = _bacc.get_activation_tables

    @functools.cache
    def filtered(arch):
        t = orig(arch)
        return {
            name: (s if name == "natural_log_exp_and_others" else set())
            for name, s in t.items()
        }

    return orig, filtered


def _build_nc():
    nc = bacc.Bacc(
        "TRN2",
        target_bir_lowering=False,
        debug=False,
        enable_asserts=False,
        num_devices=NCORES,
    )

    x8_d = nc.dram_tensor("x8", [BPC, NFT, 128, CT, FT], FP8, kind="ExternalInput").ap()
    xt_d = nc.dram_tensor("xt", [BPC, NCH, 128, NTC, CS], BF16, kind="ExternalInput").ap()
    wk8_d = nc.dram_tensor("wk8", [128, CT, C], FP8, kind="ExternalInput").ap()
    wk2_d = nc.dram_tensor("wk2", [128, CT, C], BF16, kind="ExternalInput").ap()
    wq_d = nc.dram_tensor("wqT", [128, CT, C], BF16, kind="ExternalInput").ap()
    wv_d = nc.dram_tensor("wvT", [128, CT, C], BF16, kind="ExternalInput").ap()
    wp_d = nc.dram_tensor("wpT", [128, CT, C], BF16, kind="ExternalInput").ap()
    aux_d = nc.dram_tensor("aux", [128, CT, 8], F32, kind="ExternalInput").ap()
    z_d = nc.dram_tensor("z", [C, BPC], F32, kind="ExternalOutput").ap()

    with tile.TileContext(nc) as tc:
        _emit(nc, tc, x8_d, xt_d, wk8_d, wk2_d, wq_d, wv_d, wp_d, aux_d, z_d)
    import concourse.bacc as _bacc
    orig, filtered = _act_table_filter()
    _bacc.get_activation_tables = filtered
    try:
        nc.compile()
    finally:
        _bacc.get_activation_tables = orig
    return nc


def _emit(nc, tc, x8_d, xt_d, wk8_d, wk2_d, wq_d, wv_d, wp_d, aux_d, z_d):
    from contextlib import ExitStack

    ctx = ExitStack()
    with ctx:
        const = ctx.enter_context(tc.tile_pool(name="const", bufs=1))
        x8_pool = ctx.enter_context(tc.tile_pool(name="x8", bufs=3))
        k2_pool = ctx.enter_context(tc.tile_pool(name="k2", bufs=3))
        xt_pool = ctx.enter_context(tc.tile_pool(name="xt", bufs=3))
        small = ctx.enter_context(tc.tile_pool(name="small", bufs=4))
        at_pool = ctx.enter_context(tc.tile_pool(name="at", bufs=5))
        kp2_pool = ctx.enter_context(tc.tile_pool(name="kp2", bufs=2, space="PSUM"))
        ss_pool = ctx.enter_context(tc.tile_pool(name="ss", bufs=2, space="PSUM"))
        pp_pool = ctx.enter_context(tc.tile_pool(name="pp", bufs=2, space="PSUM"))

        # ---- startup DMAs: wk8 leads the sync queue, x8 tile 0 leads the
        # scalar queue, so the first K-proj matmul group has both operands
        # as early as possible.  All other x8 tiles ride sync; the big
        # token-major chunks ride gpsimd/SWDGE exclusively.
        wk8_sb = const.tile([128, CT, C], FP8)
        nc.sync.dma_start(wk8_sb[:, 0:2, :], wk8_d[:, 0:2, :])
        nc.sync.dma_start(wk8_sb[:, 2:6, :], wk8_d[:, 2:6, :])
        x8_first = x8_pool.tile([128, CT, FT], FP8, name="x8_0_0", tag="x8")
        nc.scalar.dma_start(x8_first[:, 0:2, :], x8_d[0, 0, :, 0:2, :])
        nc.scalar.dma_start(x8_first[:, 2:6, :], x8_d[0, 0, :, 2:6, :])
        aux_sb = const.tile([128, CT, 8], F32)
        nc.scalar.dma_start(aux_sb, aux_d)
        wq_sb = const.tile([128, CT, C], BF16)
        nc.scalar.dma_start(wq_sb, wq_d)
        wk2_sb = const.tile([128, CT, C], BF16)
        nc.scalar.dma_start(wk2_sb, wk2_d)
        y_sb = aux_sb[:, :, 4:6]

        id128_bf = const.tile([128, 128], BF16)
        make_identity(nc, id128_bf)
        id32_f = const.tile([32, 32], F32)
        make_identity(nc, id32_f)

        # head indicator masks: mask32_f[c, h] = 1 if channel c belongs to
        # head h (columns padded to 32 for fp8-dual ldweights alignment)
        ones8 = const.tile([128, CT, 32], FP8)
        mask32_f = const.tile([128, CT, 32], F32)
        ones_bf = const.tile([128, CT, HEADS], BF16)
        onesT_bf = const.tile([HEADS, C], BF16)
        nc.vector.memset(ones8, 0.0)
        nc.vector.memset(mask32_f, 0.0)
        nc.vector.memset(ones_bf, 0.0)
        for c in range(CT):
            for half in range(2):
                h = 2 * c + half
                rows = slice(64 * half, 64 * (half + 1))
                nc.vector.memset(ones8[rows, c, h : h + 1], 1.0)
                nc.vector.memset(mask32_f[rows, c, h : h + 1], 1.0)
                nc.vector.memset(ones_bf[rows, c, h : h + 1], 1.0)

        # ---- statics --------------------------------------------------------
        scores_ch = {}
        pooledT_b = [const.tile([32, C], F32, name=f"pooledT{b}") for b in range(BPC)]
        pooled_sb = const.tile([128, CT, BPC * HEADS], BF16)
        outv_sb = const.tile([128, CT, BPC], BF16)

        wtld8 = const.tile([128, CT, 32 * BPC], FP8)
        qbk_sb = const.tile([32 * BPC, 1], F32)

        def qpath():
            y_bf = const.tile([128, CT, BPC], BF16)
            nc.vector.tensor_copy(out=y_bf, in_=y_sb)
            for c in range(CT):
                otp = pp_pool.tile([HEADS, 128], BF16, tag="pp")
                nc.tensor.transpose(otp, ones_bf[:, c, :], id128_bf)
                nc.vector.tensor_copy(out=onesT_bf[:, ts(c, 128)], in_=otp)
            q_sb = const.tile([128, CT, BPC], F32)
            for o in range(CT):
                qp = ss_pool.tile([128, BPC], F32, tag="ss")
                for c in range(CT):
                    nc.tensor.matmul(
                        qp, wq_sb[:, c, ts(o, 128)], y_bf[:, c, :],
                        start=(c == 0), stop=(c == CT - 1),
                    )
                nc.vector.tensor_tensor(
                    out=q_sb[:, o, :], in0=qp,
                    in1=aux_sb[:, o, 0:1].to_broadcast((128, BPC)), op=OP.add,
                )
            q2_sb = const.tile([128, CT, BPC], F32)
            nc.vector.tensor_tensor(out=q2_sb, in0=q_sb, in1=q_sb, op=OP.mult)
            ssqq = ss_pool.tile([HEADS, BPC], F32, tag="ss")
            for c in range(CT):
                nc.tensor.matmul(
                    ssqq, mask32_f[:, c, 0:HEADS], q2_sb[:, c, :],
                    start=(c == 0), stop=(c == CT - 1),
                )
            rq = const.tile([HEADS, BPC], F32)
            nc.scalar.activation(out=rq, in_=ssqq, func=AF.Ln)
            nc.scalar.activation(out=rq, in_=rq, func=AF.Exp, scale=-0.5)
            nc.vector.tensor_scalar_min(rq, rq, 1.0 / EPS)
            rq_bf = const.tile([HEADS, BPC], BF16)
            nc.vector.tensor_copy(out=rq_bf, in_=rq)
            rqbc = ss_pool.tile([128, CT, BPC], F32, tag="ss")
            for c in range(CT):
                nc.tensor.matmul(
                    rqbc[:, c, :], onesT_bf[:, ts(c, 128)], rq_bf,
                    start=(c == 0), stop=(c == CT - 1), skip_group_check=True,
                )
            qn_sb = const.tile([128, CT, BPC], F32)
            nc.vector.tensor_tensor(out=qn_sb, in0=q_sb, in1=rqbc, op=OP.mult)
            # block-diagonal placement: qbd[:, :, 32b+h] = qn[:, :, b] * mask_h
            qbd_f = const.tile([128, CT, 32 * BPC], F32)
            for b in range(BPC):
                nc.vector.tensor_tensor(
                    out=qbd_f[:, :, 32 * b : 32 * b + 32],
                    in0=qn_sb[:, :, b : b + 1].to_broadcast((128, CT, 32)),
                    in1=mask32_f, op=OP.mult,
                )
            qbd_bf = const.tile([128, CT, 32 * BPC], BF16)
            nc.vector.tensor_copy(out=qbd_bf, in_=qbd_f)
            # fold q into the K projection: raw = ((32Wk)^T Qbd)^T x + Qbd^T (32bk)
            for m in range(CT):
                wtp = ss_pool.tile([128, 32 * BPC], F32, tag="ss")
                for ot in range(CT):
                    nc.tensor.matmul(
                        wtp, wk2_sb[:, ot, ts(m, 128)], qbd_bf[:, ot, :],
                        start=(ot == 0), stop=(ot == CT - 1),
                    )
                nc.vector.tensor_copy(out=wtld8[:, m, :], in_=wtp)
            qbkp = ss_pool.tile([32 * BPC, 1], F32, tag="ss")
            for ot in range(CT):
                nc.tensor.matmul(
                    qbkp, qbd_f[:, ot, :], aux_sb[:, ot, 1:2],
                    start=(ot == 0), stop=(ot == CT - 1),
                )
            nc.vector.tensor_copy(out=qbk_sb, in_=qbkp)

        # ---- per-batch pass A ----------------------------------------------
        x8_t = {(0, 0): x8_first}
        k2_t = {}
        xt_t = {}
        att_t = {}
        rse_b = [None] * BPC
        pp_b = [None] * BPC

        def kpart(b, i):
            if (b, i) not in x8_t:
                x8 = x8_pool.tile([128, CT, FT], FP8, name=f"x8_{b}_{i}", tag="x8")
                nc.sync.dma_start(x8, x8_d[b, i])
                x8_t[(b, i)] = x8
            x8 = x8_t[(b, i)]
            # stage the token-major chunk for pooling on the SWDGE ring
            if i == 0 or i % 2 == 1:
                ch = 0 if i == 0 else (i + 1) // 2
                if ch < NCH:
                    xtc = xt_pool.tile([128, NTC, CS], BF16,
                                       name=f"xt{b}_{ch}", tag="xt")
                    nc.gpsimd.dma_start(xtc, xt_d[b, ch])
                    xt_t[(b, ch)] = xtc
            k2sb = k2_pool.tile([128, CT, FT], FP8, name=f"k2_{b}_{i}", tag="k2")
            k2_t[(b, i)] = k2sb
            for op in range(CT // 2):
                kp2 = kp2_pool.tile([128, 2, FT], F32, tag="kp2")
                for oo in range(2):
                    o = 2 * op + oo
                    for j in range(CP):
                        nc.tensor.matmul(
                            kp2[:, oo, :],
                            wk8_sb[:, 2 * j : 2 * j + 2, ts(o, 128)],
                            x8[:, 2 * j : 2 * j + 2, :],
                            start=(j == 0), stop=(j == CP - 1), perf_mode=DR,
                        )
                if HAS_BK:
                    # general path: per-o bias before the square
                    for oo in range(2):
                        o = 2 * op + oo
                        nc.scalar.activation(
                            out=k2sb[:, o, :], in_=kp2[:, oo, :], func=AF.Square,
                            bias=aux_sb[:, o, 3:4], scale=1.0 / WKS,
                        )
                else:
                    # bk == 0: one batched square over both banks
                    nc.scalar.activation(
                        out=k2sb[:, 2 * op : 2 * op + 2, :], in_=kp2,
                        func=AF.Square, scale=1.0 / WKS,
                    )

        def spart(b, i):
            R = slice(32 * b, 32 * b + HEADS)
            x8 = x8_t.pop((b, i))
            k2sb = k2_t.pop((b, i))
            sp = ss_pool.tile([32 * BPC, FT], F32, tag="ss")
            for j in range(CP):
                nc.tensor.matmul(
                    sp, wtld8[:, 2 * j : 2 * j + 2, :],
                    x8[:, 2 * j : 2 * j + 2, :],
                    start=(j == 0), stop=(j == CP - 1), perf_mode=DR,
                )
            sq = ss_pool.tile([32, FT], F32, tag="ss")
            for j in range(CP):
                nc.tensor.matmul(
                    sq, ones8[:, 2 * j : 2 * j + 2, :],
                    k2sb[:, 2 * j : 2 * j + 2, :],
                    start=(j == 0), stop=(j == CP - 1), perf_mode=DR,
                )
            rt = small.tile([HEADS, FT], F32, tag="rt", bufs=3)
            # rt = (WKS^2 * ssq)^-0.5 = 1/(WKS*||k||); cancels sp's WKS scale
            nc.scalar.activation(out=rt, in_=sq[0:HEADS, :], func=AF.Ln,
                                 scale=WKS * WKS)
            nc.scalar.activation(out=rt, in_=rt, func=AF.Exp, scale=-0.5)
            nc.vector.tensor_scalar_min(rt, rt, 1.0 / EPS)
            nc.vector.tensor_scalar(
                out=sp[R, :], in0=sp[R, :],
                scalar1=qbk_sb[R], scalar2=None, op0=OP.add,
            )
            scores_ch[(b, i)] = small.tile(
                [44, FT], F32, tag="sch", name=f"sch{b}_{i}", bufs=4)
            nc.vector.tensor_tensor(
                out=scores_ch[(b, i)][R, :],
                in0=sp[R, :], in1=rt, op=OP.mult,
            )

        neg1 = const.tile([64, 1], F32)
        nc.vector.memset(neg1, -1.0)

        def exp_ft(b, i):
            # scores are cosines in [-1, 1]: exp(s - 1) is stable without a
            # running max, so the softmax pipeline runs inside pass A.  The
            # softmax normalizer falls out of the pooling matmul (ones col).
            R = slice(32 * b, 32 * b + HEADS)
            abt = at_pool.tile(
                [64, FT], BF16, tag="ab", name=f"ab{b}_{i}", bufs=4)
            nc.scalar.activation(
                out=abt[R, :], in_=scores_ch.pop((b, i))[R, :], func=AF.Exp,
                bias=neg1[R], scale=1.0,
            )
            att = at_pool.tile(
                [128, FT // 128, 32], BF16, tag="attnT", name=f"att{b}_{i}",
                bufs=4)
            nc.sync.dma_start_transpose(att, abt[32 * b : 32 * b + 32, :])
            att_t[(b, i)] = att

        def pool_ft(b, i):
            att = att_t.pop((b, i))
            xtc = xt_t[(b, i // 2)]
            if i % 2 == 1:
                xt_t.pop((b, i // 2))
            if i == 0:
                pp0 = pp_pool.tile([HEADS, 384], F32, tag="pp", name=f"pp0_{b}")
                pp1 = pp_pool.tile([HEADS, 385], F32, tag="pp", name=f"pp1_{b}")
                pp_b[b] = (pp0, pp1)
            pp0, pp1 = pp_b[b]
            for t in range(FT // 128):
                nt = (i % 2) * (FT // 128) + t
                atl = att[:, t, 0:HEADS]
                first = i == 0 and t == 0
                last = i == NFT - 1 and t == FT // 128 - 1
                nc.tensor.matmul(
                    pp0, atl, xtc[:, nt, 0:384],
                    start=first, stop=last, skip_group_check=True,
                )
                nc.tensor.matmul(
                    pp1, atl, xtc[:, nt, 384:769],
                    start=first, stop=last, skip_group_check=True,
                )

        def pool_fin(b):
            # pp1's last column is sum(exp); normalize both pooled halves
            rse = small.tile([HEADS, 1], F32, tag="st", name=f"rse{b}")
            pp0, pp1 = pp_b[b]
            nc.vector.reciprocal(rse, pp1[:, 384:385])
            rse_b[b] = rse
            nc.vector.tensor_scalar_mul(pooledT_b[b][0:HEADS, 0:384], pp0, rse)
            nc.vector.tensor_scalar_mul(
                pooledT_b[b][0:HEADS, 384:768], pp1[:, 0:384], rse)

        wv_sb = const.tile([128, CT, C], BF16)
        wp_sb = const.tile([128, CT, C], BF16)

        def tail_b(b):
            # pooledT[12, C] -> channel-major pooled_sb columns for batch b,
            # then the Wv GEMM + head-diagonal selection for this batch.
            for c in range(CT):
                tpp = ss_pool.tile([128, 32], F32, tag="ss", name=f"tp{b}_{c}")
                nc.tensor.transpose(tpp, pooledT_b[b][:, ts(c, 128)], id32_f)
                nc.vector.tensor_copy(
                    out=pooled_sb[:, c, b * HEADS : (b + 1) * HEADS],
                    in_=tpp[:, 0:HEADS])
            for o in range(CT):
                vp = ss_pool.tile([128, HEADS], F32, tag="ss", name=f"vp{b}_{o}")
                for c in range(CT):
                    nc.tensor.matmul(
                        vp, wv_sb[:, c, ts(o, 128)],
                        pooled_sb[:, c, b * HEADS : (b + 1) * HEADS],
                        start=(c == 0), stop=(c == CT - 1),
                    )
                for half in range(2):
                    h = 2 * o + half
                    rows = slice(64 * half, 64 * (half + 1))
                    nc.vector.tensor_copy(
                        out=outv_sb[rows, o, b : b + 1],
                        in_=vp[rows, h : h + 1],
                    )

        # ---- schedule -------------------------------------------------------
        kpart(0, 0)
        kpart(0, 1)
        kpart(0, 2)
        qpath()
        for i in range(NFT):
            spart(0, i)
            exp_ft(0, i)
            if i >= 1:
                pool_ft(0, i - 1)
            if i + 3 < NFT:
                kpart(0, i + 3)
        kpart(1, 0)
        nc.scalar.dma_start(wv_sb, wv_d)
        nc.scalar.dma_start(wp_sb, wp_d)
        pool_ft(0, NFT - 1)
        pool_fin(0)
        kpart(1, 1)
        kpart(1, 2)
        tail_b(0)
        for i in range(NFT):
            spart(1, i)
            exp_ft(1, i)
            if i >= 1:
                pool_ft(1, i - 1)
            if i + 3 < NFT:
                kpart(1, i + 3)
        pool_ft(1, NFT - 1)
        pool_fin(1)
        tail_b(1)

        # ---- final Wp GEMM + bias + output ---------------------------------
        z_sb = const.tile([128, CT, BPC], F32)
        for o2 in range(CT):
            zp = ss_pool.tile([128, BPC], F32, tag="ss")
            for o in range(CT):
                nc.tensor.matmul(
                    zp, wp_sb[:, o, ts(o2, 128)], outv_sb[:, o, :],
                    start=(o == 0), stop=(o == CT - 1),
                )
            nc.vector.tensor_tensor(
                out=z_sb[:, o2, :], in0=zp,
                in1=aux_sb[:, o2, 2:3].to_broadcast((128, BPC)), op=OP.add,
            )
        nc.sync.dma_start(z_d.rearrange("(c p) b -> p c b", p=128), z_sb)


_NC_CACHE = None


def _get_nc():
    global _NC_CACHE
    if _NC_CACHE is None:
        _NC_CACHE = _build_nc()
    return _NC_CACHE


def make_in_maps(inputs):
    x = np.ascontiguousarray(np.asarray(inputs["x"], dtype=np.float32)).reshape(B, C, N)
    y = np.asarray(inputs["y"], dtype=np.float32).reshape(B, C)
    Wq = np.asarray(inputs["Wq"], dtype=np.float32)
    bq = np.asarray(inputs["bq"], dtype=np.float32)
    Wkv = np.asarray(inputs["Wkv"], dtype=np.float32)
    bkv = np.asarray(inputs["bkv"], dtype=np.float32)
    Wp = np.asarray(inputs["Wp"], dtype=np.float32)
    bp = np.asarray(inputs["bp"], dtype=np.float32)

    wk, wv = Wkv[:C], Wkv[C:]
    bk, bv = bkv[:C], bkv[C:]

    def ptile(wT, dt=ml_dtypes.bfloat16):
        # [C, C] (contraction-major) -> [128, CT, C] SBUF layout
        return np.ascontiguousarray(
            wT.reshape(CT, 128, C).transpose(1, 0, 2)).astype(dt)

    wk8 = ptile(wk.T * WKS, ml_dtypes.float8_e4m3)
    wk2 = ptile(wk * WKS)
    wqT = ptile(Wq.T)
    wvT = ptile(wv.T)
    wpT = ptile(Wp.T)
    bpz = (Wp @ bv + bp).astype(np.float32)

    # channel-major fp8 x: [B, NFT, 128, CT, FT], channel = ct*128 + p
    x8 = np.ascontiguousarray(
        x.reshape(B, CT, 128, NFT, FT).transpose(0, 3, 2, 1, 4)
    ).astype(ml_dtypes.float8_e4m3)
    # token-major bf16 x + ones col: [B, NCH, 128, NTC, CS]
    xt = np.ones((B, NCH, 128, NTC, CS), ml_dtypes.bfloat16)
    xt[..., :C] = x.reshape(B, C, NCH, NTC, 128).transpose(
        0, 2, 4, 3, 1).astype(ml_dtypes.bfloat16)

    def pcol(v):
        return v.reshape(CT, 128).T  # [(c p)] -> [p, c]

    in_maps = []
    for i in range(NCORES):
        aux = np.zeros((128, CT, 8), np.float32)
        aux[:, :, 0] = pcol(bq)
        aux[:, :, 1] = pcol(bk * WKS)
        aux[:, :, 2] = pcol(bpz)
        aux[:, :, 3] = pcol(bk)
        yb = y[i * BPC : (i + 1) * BPC]  # [2, C]
        for b in range(BPC):
            aux[:, :, 4 + b] = pcol(yb[b])
        in_maps.append({
            "x8": np.ascontiguousarray(x8[i * BPC : (i + 1) * BPC]),
            "xt": np.ascontiguousarray(xt[i * BPC : (i + 1) * BPC]),
            "wk8": wk8, "wk2": wk2, "wqT": wqT, "wvT": wvT, "wpT": wpT,
            "aux": aux,
        })
    return in_maps


def kernel(**inputs):
    global HAS_BK
    HAS_BK = bool(np.any(np.asarray(inputs["bkv"], dtype=np.float32)[:C]))
    nc = _get_nc()
    in_maps = make_in_maps(inputs)
    res = run_bass_kernel_spmd(nc, in_maps, core_ids=list(range(NCORES)))
    z = np.concatenate([r["z"].T for r in res.results], axis=0)
    return z.reshape(B, C, 1, 1).astype(np.float32)
